# revision 35
# baseline (speedup 1.0000x reference)
import numpy as np

import concourse.bass as bass
import concourse.bacc as bacc
import concourse.mybir as mybir
import concourse.tile as tile
from concourse.bass_utils import run_bass_kernel_spmd
from concourse.masks import make_identity

FP = mybir.dt.float32
BF = mybir.dt.bfloat16
U16 = mybir.dt.uint16
AF = mybir.ActivationFunctionType
OP = mybir.AluOpType

GRID = 32
NN = 1024
F_IN = 32
H = 256
B = 64
S = 8
NCORE = 8
OBS_W = NN + NN * F_IN
MIN_VAL = -10000000.0
EPS_LN = 1e-5
EPS_BN = 1e-5
PAD = 32
HW = NN + 2 * PAD

USE_BF16 = True
PROFILE = False
LAST_EXEC_NS = None
TRACE_KWARGS = {}

_ACT_PATCHED = False


def _patch_act_tables():
    global _ACT_PATCHED
    if _ACT_PATCHED:
        return
    import concourse.hw_specs as hw_specs

    orig = hw_specs.get_activation_tables

    def patched(arch):
        t = orig(arch)
        if "natural_log_exp_and_others" in t:
            both = t["natural_log_exp_and_others"]
            if AF.Ln in both and AF.Exp in both:
                for name, fns in t.items():
                    if name != "natural_log_exp_and_others":
                        fns.discard(AF.Exp)
                        fns.discard(AF.Ln)
        return t

    hw_specs.get_activation_tables = patched
    for mod in (bacc, bass):
        if getattr(mod, "get_activation_tables", None) is orig:
            mod.get_activation_tables = patched
    _ACT_PATCHED = True


def _build(cfg, unused_b2=None, unused_bf16=None) -> bass.Bass:
    has_gin_bias = cfg["has_gin_bias"]
    fast_ln = cfg["fast_ln"]
    gscal = cfg["gscal"]
    b2_val = cfg["b2_val"]

    _patch_act_tables()
    nc = bacc.Bacc("TRN2", target_bir_lowering=False, debug=False)

    obs = nc.declare_dram_parameter("obs", [S, OBS_W], FP, isOutput=False)
    w0 = nc.declare_dram_parameter("w0", [96, H], U16, isOutput=False)
    ws = nc.declare_dram_parameter("ws", [3, H, H], U16, isOutput=False)
    w1x = nc.declare_dram_parameter("w1x", [F_IN, 512], U16, isOutput=False)
    w1h = nc.declare_dram_parameter("w1h", [8, 128, 512], U16, isOutput=False)
    w2 = nc.declare_dram_parameter("w2", [4, 128], U16, isOutput=False)
    bns = nc.declare_dram_parameter("bns", [512], FP, isOutput=False)
    bnt = nc.declare_dram_parameter("bnt", [512], FP, isOutput=False)
    if has_gin_bias:
        gbias = nc.declare_dram_parameter("gbias", [4, H], U16, isOutput=False)
    if not fast_ln:
        grows = nc.declare_dram_parameter("grows", [4, H], FP, isOutput=False)
        bbr = nc.declare_dram_parameter("bbr", [4, H], FP, isOutput=False)
    y_out = nc.declare_dram_parameter("y", [S, NN], FP, isOutput=True)

    from contextlib import ExitStack

    with tile.TileContext(nc) as tc, ExitStack() as ctx:
        wp = ctx.enter_context(tc.tile_pool(name="w", bufs=1))
        px = ctx.enter_context(tc.tile_pool(name="px", bufs=2))
        pxs = ctx.enter_context(tc.tile_pool(name="pxs", bufs=9))
        ph = ctx.enter_context(tc.tile_pool(name="ph", bufs=17))
        phh = ctx.enter_context(tc.tile_pool(name="phh", bufs=3))
        pzsq = ctx.enter_context(tc.tile_pool(name="pzsq", bufs=3))
        plnv = ctx.enter_context(tc.tile_pool(name="plnv", bufs=3))
        psgs = ctx.enter_context(tc.tile_pool(name="psgs", bufs=3))
        pzsb = ctx.enter_context(tc.tile_pool(name="pzsb", bufs=2))
        pfin = ctx.enter_context(tc.tile_pool(name="pfin", bufs=2))
        pz = ctx.enter_context(tc.tile_pool(name="pz", bufs=6, space="PSUM"))
        pvar = ctx.enter_context(tc.tile_pool(name="pvar", bufs=2, space="PSUM"))

        ident = wp.tile([128, 128], FP, tag="id")
        make_identity(nc, ident[:])

        ones_mat = wp.tile([128, 128], BF, tag="ones_mat")
        nc.gpsimd.memset(ones_mat[:].bitcast(U16), 0x3F80)
        ones_row = wp.tile([1, 512], BF, tag="ones_row")
        nc.gpsimd.memset(ones_row[:].bitcast(U16), 0x3F80)

        sel_col = wp.tile([128, 1], FP, tag="sel_col")
        nc.gpsimd.memset(sel_col[:], 0.0)
        for q in range(4):
            nc.gpsimd.memset(sel_col[32 * q: 32 * q + 1, :], 1.0)

        zero_col = wp.tile([128, 1], FP, tag="zero_col")
        nc.gpsimd.memset(zero_col[:], 0.0)
        eps_col = wp.tile([128, 4], FP, tag="eps_col")
        for l in range(4):
            gl = gscal[l] if fast_ln else 1.0
            nc.gpsimd.memset(eps_col[:, l: l + 1], EPS_LN / (gl * gl))

        w0_sb = wp.tile([96, H], BF, tag="w0")
        nc.scalar.dma_start(w0_sb[:].bitcast(U16), w0[:, :])

        wl_sb = wp.tile([128, 3 * 512], BF, tag="wl")
        for l in range(3):
            nc.scalar.dma_start(
                wl_sb[:, l * 512:(l + 1) * 512].bitcast(U16)
                .rearrange("p (ci co c) -> p ci co c", ci=2, co=2),
                ws[l].rearrange("(ci p) (co c) -> p ci co c", p=128, c=128),
            )

        w1x_sb = wp.tile([F_IN, 512], BF, tag="w1x")
        nc.scalar.dma_start(w1x_sb[:, :].bitcast(U16), w1x[:, :])
        w1h_sb = wp.tile([128, 8 * 512], BF, tag="w1h")
        nc.scalar.dma_start(
            w1h_sb[:].bitcast(U16).rearrange("p (j m) -> p j m", j=8),
            w1h[:, :, :].rearrange("j p m -> p j m"),
        )
        w2_sb = wp.tile([128, 4], BF, tag="w2")
        nc.scalar.dma_start(w2_sb[:].bitcast(U16), w2[:, :].rearrange("k p -> p k"))

        bns_sb = wp.tile([128, 4], FP, tag="bns")
        nc.scalar.dma_start(bns_sb[:], bns[:].rearrange("(m p) -> p m", p=128))
        bnt_sb = wp.tile([128, 4], FP, tag="bnt")
        nc.scalar.dma_start(bnt_sb[:], bnt[:].rearrange("(m p) -> p m", p=128))

        if has_gin_bias:
            gb_row = wp.tile([1, 4 * H], BF, tag="gb")
            nc.scalar.dma_start(
                gb_row[:].bitcast(U16).rearrange("q (l n) -> q l n", l=4),
                gbias[:, :],
            )
        if not fast_ln:
            g_col = wp.tile([128, 8], FP, tag="gcol")
            nc.scalar.dma_start(
                g_col[:].rearrange("p (l c) -> p l c", c=2),
                grows[:, :].rearrange("l (c p) -> p l c", p=128),
            )
            bb_sb = wp.tile([128, 8], FP, tag="bb")
            nc.scalar.dma_start(
                bb_sb[:].rearrange("p (l c) -> p l c", c=2),
                bbr[:, :].rearrange("l (c p) -> p l c", p=128),
            )

        warm_ps = pz.tile([128, 128], FP, tag="z", name="warm_ps")
        for i in range(30):
            nc.tensor.matmul(
                warm_ps[:, :], ones_mat[:, :], ones_mat[:, :],
                start=(i == 0), stop=(i == 29),
            )

        def wchunk(l, ci, co):
            if l == 0:
                return w0_sb[:, co * 128: co * 128 + 128]
            return wl_sb[:, (l - 1) * 512 + ci * 256 + co * 128
                         : (l - 1) * 512 + ci * 256 + co * 128 + 128]

        def build_hh(hh_ap, src_ap):
            sv = src_ap.rearrange("p (r c) -> p r c", c=GRID)
            dv = hh_ap.rearrange("p (r c) -> p r c", c=GRID)
            nc.gpsimd.tensor_add(dv[:, :, 1:31], sv[:, :, 0:30], sv[:, :, 2:32])
            nc.gpsimd.tensor_copy(dv[:, :, 0:1], sv[:, :, 1:2])
            nc.gpsimd.tensor_copy(dv[:, :, 31:32], sv[:, :, 30:31])

        def prep_x(s):
            x_nm = px.tile([128, 256], FP, tag="xnm")
            nc.sync.dma_start(
                x_nm[:].rearrange("p (b f) -> p b f", f=F_IN),
                obs[s, NN:OBS_W].rearrange("(b p f) -> p b f", p=128, f=F_IN),
            )
            xs = pxs.tile([96, HW], BF, tag="xs")
            nc.gpsimd.memset(xs[0:32, 32: 64].bitcast(U16), 0)
            nc.gpsimd.memset(xs[64:96, NN: NN + 32].bitcast(U16), 0)
            for half in range(2):
                x_tfm = pz.tile([F_IN, 512], FP, tag="z", name="xtf")
                for i in range(4):
                    b = half * 4 + i
                    nc.tensor.transpose(
                        x_tfm[:, i * 128:(i + 1) * 128],
                        x_nm[:, b * F_IN:(b + 1) * F_IN],
                        ident[:],
                    )
                nc.vector.tensor_copy(
                    xs[0:32, 2 * PAD + half * 512: 2 * PAD + half * 512 + 512],
                    x_tfm[:],
                )
                nc.vector.tensor_copy(
                    xs[64:96, half * 512: half * 512 + 512], x_tfm[:]
                )
            build_hh(xs[32:64, PAD: PAD + NN], xs[0:32, 2 * PAD: 2 * PAD + NN])
            return {"s": s, "xs": xs, "h": []}

        def layer_mm(st, l):
            kc = 1 if l == 0 else 2
            st["zsq"] = pzsq.tile([128, 2048], BF, tag="zsq", name="zsq")
            h_t = ph.tile([128, 2 * HW], BF, tag="ht", name="ht")
            nc.gpsimd.memset(h_t[:, 0:PAD].bitcast(U16), 0)
            nc.gpsimd.memset(h_t[:, PAD + NN: HW + PAD].bitcast(U16), 0)
            nc.gpsimd.memset(h_t[:, HW + PAD + NN: 2 * HW].bitcast(U16), 0)
            st["h"].append(h_t)
            zz = [[None, None], [None, None]]
            for half in range(2):
                toff = half * 512
                for co in range(2):
                    z = pz.tile([128, 512], FP, tag="z")
                    if l == 0:
                        nmm = 1 + (1 if has_gin_bias else 0)
                        nc.tensor.matmul(
                            z[:, :], wchunk(0, 0, co),
                            st["xs"][:, PAD + toff: PAD + toff + 512],
                            start=True, stop=(nmm == 1),
                        )
                    else:
                        nmm = 3 * kc + (1 if has_gin_bias else 0)
                        i = 0
                        for ci in range(kc):
                            hprev = st["h"][l - 1]
                            hhprev = st["hh"]
                            hh_w = hhprev[:, ci * NN + toff: ci * NN + toff + 512]
                            up_w = hprev[:, ci * HW + toff: ci * HW + toff + 512]
                            dn_w = hprev[:, ci * HW + 2 * PAD + toff
                                         : ci * HW + 2 * PAD + toff + 512]
                            for rhs in (hh_w, up_w, dn_w):
                                nc.tensor.matmul(
                                    z[:, :], wchunk(l, ci, co), rhs,
                                    start=(i == 0), stop=(i == nmm - 1),
                                )
                                i += 1
                    if has_gin_bias:
                        nc.tensor.matmul(
                            z[:, :],
                            gb_row[0:1, l * H + co * 128: l * H + co * 128 + 128],
                            ones_row[0:1, 0:512],
                            start=False, stop=True,
                        )
                    zz[half][co] = z
                for co in range(2):
                    nc.scalar.activation(
                        st["zsq"][:, co * NN + toff: co * NN + toff + 512],
                        zz[half][co][:, :], AF.Square, bias=zero_col[:, 0:1],
                    )
            st["zz"] = zz

        def layer_ln_half(st, l, half):
            toff = half * 512
            zz = st["zz"]
            zsq = st["zsq"]
            h_t = st["h"][l]
            var = pvar.tile([128, 512], FP, tag="var")
            for co in range(2):
                nc.tensor.matmul(
                    var[:, :], ones_mat[:, :],
                    zsq[:, co * NN + toff: co * NN + toff + 512],
                    start=(co == 0), stop=(co == 1),
                )
            lnv = plnv.tile([128, 512], FP, tag="lnv")
            g = gscal[l] if fast_ln else 1.0
            nc.scalar.activation(
                lnv[:], var[:, :], AF.Ln,
                bias=eps_col[:, l: l + 1], scale=1.0 / (H * g * g),
            )
            sgs = psgs.tile([128, 512], BF, tag="sgs")
            nc.scalar.activation(
                sgs[:], lnv[:], AF.Exp, bias=zero_col[:, 0:1], scale=-0.5
            )
            for co in range(2):
                if fast_ln:
                    nc.vector.scalar_tensor_tensor(
                        h_t[:, co * HW + PAD + toff: co * HW + PAD + toff + 512],
                        zz[half][co][:, :], 0.0, sgs[:],
                        op0=OP.max, op1=OP.mult,
                    )
                else:
                    tmb = psgs.tile([128, 512], BF, tag="sgs", name="tmb")
                    nc.vector.scalar_tensor_tensor(
                        tmb[:], zz[half][co][:, :],
                        g_col[:, l * 2 + co: l * 2 + co + 1], sgs[:],
                        op0=OP.mult, op1=OP.mult,
                    )
                    nc.vector.tensor_scalar(
                        out=h_t[:, co * HW + PAD + toff
                                : co * HW + PAD + toff + 512],
                        in0=tmb[:],
                        scalar1=bb_sb[:, l * 2 + co: l * 2 + co + 1],
                        scalar2=0.0,
                        op0=OP.add, op1=OP.max,
                    )

        def layer_fin(st, l):
            st.pop("zz")
            st.pop("zsq")
            if l < 3:
                h_t = st["h"][l]
                hh_t = phh.tile([128, 2 * NN], BF, tag="hh")
                for co in range(2):
                    build_hh(
                        hh_t[:, co * NN:(co + 1) * NN],
                        h_t[:, co * HW + PAD: co * HW + PAD + NN],
                    )
                st["hh"] = hh_t

        def layer_round(pair, l):
            s0, s1 = pair
            layer_mm(s0, l)
            layer_ln_half(s0, l, 0)
            layer_mm(s1, l)
            layer_ln_half(s0, l, 1)
            layer_fin(s0, l)
            layer_ln_half(s1, l, 0)
            layer_ln_half(s1, l, 1)
            layer_fin(s1, l)

        def w1_chunk(st, m):
            if m == 0:
                st["zsb"] = pzsb.tile([128, 4096], BF, tag="zsb", name="zsb")
            zsb = st["zsb"]
            zw = [pz.tile([128, 512], FP, tag="z", name="zw") for _ in range(2)]
            for kc9 in range(9):
                if kc9 == 0:
                    lhsT = w1x_sb[:, m * 128:(m + 1) * 128]
                else:
                    j = kc9 - 1
                    lhsT = w1h_sb[:, j * 512 + m * 128: j * 512 + m * 128 + 128]
                for half in range(2):
                    toff = half * 512
                    if kc9 == 0:
                        rhs = st["xs"][0:32, 2 * PAD + toff: 2 * PAD + toff + 512]
                    else:
                        j = kc9 - 1
                        co = j % 2
                        rhs = st["h"][j // 2][:, co * HW + PAD + toff
                                              : co * HW + PAD + toff + 512]
                    nc.tensor.matmul(
                        zw[half][:, :], lhsT, rhs,
                        start=(kc9 == 0), stop=(kc9 == 8),
                    )
            for half in range(2):
                nc.scalar.activation(
                    zsb[:, m * NN + half * 512: m * NN + half * 512 + 512],
                    zw[half][:, :], AF.Relu,
                    scale=bns_sb[:, m: m + 1], bias=bnt_sb[:, m: m + 1],
                )

        def head_finish(st):
            s = st["s"]
            zsb = st["zsb"]
            y_s = pfin.tile([1, NN], FP, tag="ys")
            for c2 in range(2):
                yp4 = pvar.tile([128, 512], FP, tag="var", name="yp4")
                for m in range(4):
                    nc.tensor.matmul(
                        yp4[32 * m: 32 * m + 1, :], w2_sb[:, m: m + 1],
                        zsb[:, m * NN + c2 * 512: m * NN + c2 * 512 + 512],
                        start=True, stop=True, tile_position=(0, 32 * m),
                    )
                yc = plnv.tile([128, 512], FP, tag="lnv", name="yc")
                nc.scalar.copy(yc[:], yp4[:, :])
                yp = pvar.tile([1, 512], FP, tag="var", name="yp")
                nc.tensor.matmul(
                    yp[0:1, :], sel_col[:, 0:1], yc[:],
                    start=True, stop=True,
                )
                nc.vector.tensor_copy(y_s[:, c2 * 512:(c2 + 1) * 512], yp[0:1, :])
            if b2_val != 0.0:
                nc.scalar.add(y_s[:], y_s[:], b2_val)
            m_s = pfin.tile([1, NN], FP, tag="ms")
            nc.sync.dma_start(m_s[:], obs[s: s + 1, 0:NN])
            yf = pfin.tile([1, NN], FP, tag="yfin")
            nc.gpsimd.memset(yf[:], MIN_VAL)
            nc.vector.copy_predicated(yf[:], m_s[:].bitcast(mybir.dt.uint32), y_s[:])
            nc.sync.dma_start(y_out[s: s + 1, :], yf[:])

        def head_units(st):
            return [lambda m=m, st=st: w1_chunk(st, m) for m in range(4)] \
                + [lambda st=st: head_finish(st)]

        headq = []

        def flush(k=None):
            n = len(headq) if k is None else min(k, len(headq))
            for _ in range(n):
                headq.pop(0)()

        prev = [prep_x(0), prep_x(1)]
        for l in range(4):
            layer_round(prev, l)
        for p in range(1, 4):
            for st in prev:
                headq.extend(head_units(st))
            cur = [prep_x(2 * p), prep_x(2 * p + 1)]
            for l in range(4):
                layer_round(cur, l)
                flush(3)
            flush()
            prev = cur
        for st in prev:
            headq.extend(head_units(st))
        flush()

    nc.finalize()
    return nc


_BUILD_CACHE = {}
_CFG = None


def _get_nc(has_gin_bias=None, b2_val=None, use_bf16=None) -> bass.Bass:
    cfg = _CFG
    key = (cfg["has_gin_bias"], cfg["fast_ln"], cfg["gscal"], cfg["b2_val"])
    if key not in _BUILD_CACHE:
        _BUILD_CACHE[key] = _build(cfg)
    return _BUILD_CACHE[key]


def prep_maps(observations, W0, b0, g0, be0, Ws, bs, gs, bes,
              W1, b1, bn_g, bn_b, bn_m, bn_v, W2, b2, **_ignored):
    global _CFG
    obs = np.ascontiguousarray(np.asarray(observations, np.float32))
    W0 = np.asarray(W0, np.float32)
    Ws = np.asarray(Ws, np.float32)
    W1 = np.asarray(W1, np.float32)
    W2 = np.asarray(W2, np.float32)

    W0c = np.ascontiguousarray(W0 - W0.mean(axis=1, keepdims=True))
    Wsc = np.ascontiguousarray(Ws - Ws.mean(axis=2, keepdims=True))

    gg = np.stack([np.asarray(g0, np.float32)]
                  + [np.asarray(gs, np.float32)[i] for i in range(3)])
    bb = np.stack([np.asarray(be0, np.float32)]
                  + [np.asarray(bes, np.float32)[i] for i in range(3)])
    graw = np.stack([np.asarray(b0, np.float32)]
                    + [np.asarray(bs, np.float32)[i] for i in range(3)])
    gbias = np.ascontiguousarray(graw - graw.mean(axis=1, keepdims=True))
    has_gin_bias = bool(np.any(np.abs(gbias) > 0.0))

    g_const = all(np.all(gg[i] == gg[i][0]) for i in range(4))
    fast_ln = bool(np.all(bb == 0.0) and g_const
                   and all(gg[i][0] > 0 for i in range(4)))
    gscal = tuple(float(gg[i][0]) for i in range(4)) if fast_ln else (0.0,) * 4

    bn_scale = (np.asarray(bn_g, np.float32)
                / np.sqrt(np.asarray(bn_v, np.float32) + EPS_BN)).astype(np.float32)
    bn_shift = ((np.asarray(b1, np.float32) - np.asarray(bn_m, np.float32)) * bn_scale
                + np.asarray(bn_b, np.float32)).astype(np.float32)
    b2_val = float(np.asarray(b2, np.float32).reshape(-1)[0])

    def bf16(a):
        t = np.ascontiguousarray(a, np.float32).view(np.uint32)
        r = ((t + 0x7FFF + ((t >> 16) & 1)) >> 16).astype(np.uint16)
        return r

    w1x = np.ascontiguousarray(W1[:F_IN])
    w1h = np.ascontiguousarray(W1[F_IN:].reshape(8, 128, 512))
    w2r = np.ascontiguousarray(W2.reshape(4, 128))

    _CFG = {"has_gin_bias": has_gin_bias, "fast_ln": fast_ln,
            "gscal": gscal, "b2_val": b2_val}

    shared = {
        "w0": bf16(W0c), "ws": bf16(Wsc), "w1x": bf16(w1x),
        "w1h": bf16(w1h), "w2": bf16(w2r),
        "bns": bn_scale, "bnt": bn_shift,
    }
    if has_gin_bias:
        shared["gbias"] = bf16(gbias)
    if not fast_ln:
        shared["grows"] = np.ascontiguousarray(gg)
        shared["bbr"] = np.ascontiguousarray(bb)
    in_maps = []
    for c in range(NCORE):
        m = dict(shared)
        m["obs"] = np.ascontiguousarray(obs[c * S: (c + 1) * S])
        in_maps.append(m)
    return in_maps, has_gin_bias, b2_val


def kernel(**inputs) -> np.ndarray:
    global LAST_EXEC_NS
    in_maps, has_gin_bias, b2_val = prep_maps(**inputs)
    nc = _get_nc(has_gin_bias, b2_val, USE_BF16)
    res = run_bass_kernel_spmd(
        nc, in_maps, list(range(NCORE)), trace=PROFILE, **TRACE_KWARGS
    )
    LAST_EXEC_NS = res.exec_time_ns
    y = np.concatenate([res.results[c]["y"] for c in range(NCORE)], axis=0)
    return y.reshape(B, NN).astype(np.float32)
